# revision 9
# baseline (speedup 1.0000x reference)
"""Trainium2 Bass kernel for nn_Attention_90074054132266.

Full multi-head attention (B=2, S=4096, D=512, H=8, HD=64) with RoPE on
q/k, sharded over 8 NeuronCores: batch x head-pair (data parallel over
batch, tensor parallel over heads; core c handles batch c//4, heads
2*(c%4), 2*(c%4)+1). Each core computes a partial output projection
(its 2 heads' contribution); the host sums the 4 per-batch partials
(the "all-reduce") and adds wo_b.

Per-core device algorithm (everything stored transposed, bf16 matmuls):
  - host passes x[b].T; projections q^T/k^T = wq^T-chunks @ x^T as
    N=512 bf16 matmuls. x^T is resident in SBUF (loaded once).
  - RoPE via duplicated projections with half-swapped weight columns
    (q2^T[d] = q^T[(d+32)%64 per head]) + sign-baked cos/sin tables
    (bf16): q_rot = q^T * cosf + q2^T * sinf  (3 VectorE ops).
  - scores computed transposed: S^T[k-chunk, q] = K_rot slice.T @ Q_rot
    (contraction over d=64, zero-padded per head to K=128 so the HAM
    clock gate keeps the PE at 2.4 GHz). exp(S/8) on ScalarE from PSUM
    with the 1/sqrt(hd) scale folded in (no max subtraction: scores
    ~ N(0,1), exp is safe in fp32).
  - P@V accumulates O'^T[65, q] over 32 k-chunks; V' has a ones column
    so row 64 = softmax denominator Z for free.
  - output projection U_h = O_h^T.T @ wo_h per head; normalization by
    1/Z_h[q] applied as a per-partition scalar on PSUM evacuation.

Pipeline structure (the point of this revision):
  - DMAs are emitted in consumption order (w, then per 512-chunk:
    xt, cos, sin) so the first projection starts ~3us in.
  - qt0's attention is interleaved with the k/v projection: pair p of
    k-chunks only needs k/v chunks from projection chunk p//2, so
    scores/exp/PV start after the first projection chunk instead of
    after all eight.
  - PV of pair p-1 is emitted AFTER scores of pair p: the PE queue is
    in-order, so this keeps the PE busy while ScalarE runs exp(p).
"""

import os
import sys

sys.path.insert(0, "/opt/trn_rl_repo")

import numpy as np

B, S, DIM, HEADS, HD = 2, 4096, 512, 8, 64
HALF = HD // 2
NCORES = 8
HPC = 2  # heads per core
DPC = HPC * HD  # 128 projection columns per core
NSC = S // 512  # 8 column chunks of 512
NKC = S // 128  # 32 k-chunks of 128
NUT = S // 128  # 32 q-row tiles of 128
KC2 = NKC // 2  # 16 pairs of k-chunks (exp batches of [128, 1024])
VW = 2 * (HD + 1)  # 130: per-k V' row for both heads (64+1 each)

_CACHE = {}


def _split_multiwait_drains(nc):
    """The walrus build in this container rejects any instruction with
    more than one sync-wait ("Too many sync wait commands"). Hoist the
    extra waits onto preceding same-engine NoOps, leaving one wait on
    the original instruction."""
    import bass_rust
    import concourse.mybir as mybir

    for fn in nc.m.functions:
        for bb in fn.blocks:
            new_insts = []
            changed = False
            for inst in bb.instructions:
                si = getattr(inst, "sync_info", None)
                if si is not None and len(si.on_wait) > 1:
                    waits = list(si.on_wait)
                    for k, w in enumerate(waits[:-1]):
                        d = mybir.InstNoOp(name=f"{inst.name}w{k}", ins=[], outs=[])
                        d.engine = inst.engine
                        d.sync_info = bass_rust.SyncInfo(on_wait=[w], on_update=[])
                        new_insts.append(d)
                    inst.sync_info = bass_rust.SyncInfo(
                        on_wait=[waits[-1]], on_update=list(si.on_update)
                    )
                    changed = True
                new_insts.append(inst)
            if changed:
                bb.instructions = new_insts


def _build(qk_bias, v_bias):
    import concourse.bass as bass
    import concourse.tile as tile
    from concourse import mybir

    F32 = mybir.dt.float32
    BF16 = mybir.dt.bfloat16
    F16 = mybir.dt.float16
    EXP = mybir.ActivationFunctionType.Exp
    MUL = mybir.AluOpType.mult
    ADD = mybir.AluOpType.add

    nc = bass.Bass("TRN2")

    xt_e = nc.declare_dram_parameter("xt", [DIM, S], BF16, isOutput=False)
    w_e = {}
    for name in ("wq", "wqp", "wk", "wkp", "wv"):
        w_e[name] = nc.declare_dram_parameter(name, [DIM, DPC], BF16, isOutput=False)
    wo_e = nc.declare_dram_parameter("wo", [DPC, DIM], BF16, isOutput=False)
    cos_e = nc.declare_dram_parameter("cosf", [DPC, S], F16, isOutput=False)
    sin_e = nc.declare_dram_parameter("sinf", [DPC, S], F16, isOutput=False)
    b_e = {}
    if qk_bias:
        for name in ("qb", "qbp", "kb", "kbp"):
            b_e[name] = nc.declare_dram_parameter(name, [DPC, 1], F32, isOutput=False)
    if v_bias:
        b_e["vb"] = nc.declare_dram_parameter("vb", [1, DPC], F32, isOutput=False)
    z_e = nc.declare_dram_parameter("zpage", [1, 512], BF16, isOutput=False)
    out_e = nc.declare_dram_parameter("out", [S, DIM], F32, isOutput=True)

    with tile.TileContext(nc) as tc:
        with (
            tc.tile_pool(name="persist", bufs=1) as P,
            tc.tile_pool(name="work", bufs=2) as W,
            tc.tile_pool(name="pps", bufs=1, space="PSUM") as PPS,
            tc.tile_pool(name="pss", bufs=2, space="PSUM") as PSS,
            tc.tile_pool(name="pou", bufs=1, space="PSUM") as POU,
        ):
            # ---- persistent SBUF tensors ----
            qr = P.tile([DPC, S], BF16, tag="qr")  # rotated q^T
            # rotated k^T, zero-padded per head to full K=128 contraction
            # (row-masked K=64 matmuls don't count as PE-busy for the HAM
            # clock gate; mixing them with PV pins the PE at 1.2 GHz)
            krA = P.tile([DPC, S], BF16, tag="krA")
            krB = P.tile([DPC, S], BF16, tag="krB")

            def zfill(dst):
                srcap = bass.AP(
                    tensor=z_e[:].tensor,
                    offset=z_e[:].offset,
                    ap=[[0, HD], [0, NSC], [1, 512]],
                )
                nc.sync.dma_start(out=dst, in_=srcap)
            # V' rows: per k-chunk st, V[k, :] for head A cols 0:64 + ones
            # col 64, head B cols 65:129 + ones col 129.
            vb_sb = P.tile([128, NKC, VW], BF16, tag="vboth")
            # unnormalized O^T, zero-padded per head (HAM: keep K=128)
            otA = P.tile([DPC, S], BF16, tag="otA")
            otB = P.tile([DPC, S], BF16, tag="otB")
            wo_sb = P.tile([DPC, DIM], BF16, tag="wo")
            zrow = P.tile([33, S], F32, tag="zrow")
            zt = P.tile([128, 2 * NUT], F32, tag="zt")
            izt = P.tile([128, 2 * NUT], F32, tag="izt")
            # resident x^T: [p, dim-chunk, col]
            xt_sb = P.tile([128, 4, S], BF16, tag="xt")
            cos_sb = P.tile([DPC, S], F16, tag="cos")
            sin_sb = P.tile([DPC, S], F16, tag="sin")
            w_sb = {}
            for name in ("wq", "wqp", "wk", "wkp", "wv"):
                t = P.tile([128, 4, DPC], BF16, tag=name, name=f"w_{name}")
                w_sb[name] = t

            # ---- DMA emission in strict first-need order (all on the
            # SP engine: only its queues are HW-DGE; triggers serialize
            # per transfer, so order = data arrival order) ----
            def wload(name):
                nc.sync.dma_start(
                    out=w_sb[name],
                    in_=w_e[name][:].rearrange("(c p) m -> p c m", p=128),
                )

            xt_r = xt_e[:].rearrange("(c p) s -> c p s", p=128)

            def xtload(sc, w=512):
                qs_ = bass.ds(sc * 512, w)
                for c in range(4):
                    nc.sync.dma_start(out=xt_sb[:, c, qs_], in_=xt_r[c, :, qs_])

            def csload(sc, w=512):
                qs_ = bass.ds(sc * 512, w)
                nc.sync.dma_start(out=cos_sb[:, qs_], in_=cos_e[:, qs_])
                nc.sync.dma_start(out=sin_sb[:, qs_], in_=sin_e[:, qs_])

            wload("wq")
            xtload(0)
            wload("wqp")
            csload(0)
            bias_sb = {}
            if qk_bias:
                for name in ("qb", "qbp", "kb", "kbp"):
                    t = P.tile([DPC, 1], F32, tag=name, name=f"b_{name}")
                    nc.sync.dma_start(out=t, in_=b_e[name][:])
                    bias_sb[name] = t
            if v_bias:
                vbias_bc = P.tile([128, DPC], F32, tag="vbias")
                src = bass.AP(
                    tensor=b_e["vb"].tensor,
                    offset=b_e["vb"].offset,
                    ap=[[0, 128], [1, DPC]],
                )
                nc.sync.dma_start(out=vbias_bc, in_=src)

            wload("wk")
            wload("wkp")
            zfill(krA[HD:DPC, :])
            zfill(krB[0:HD, :])
            wload("wv")
            zfill(otA[HD:DPC, :])
            zfill(otB[0:HD, :])
            xtload(1)
            csload(1)
            nc.sync.dma_start(out=wo_sb, in_=wo_e[:])
            for sc in range(2, NSC, 2):
                # 1024-wide transfers: 2KB per partition line
                xtload(sc, 1024)
                csload(sc, 1024)

            # ones columns of V' (written once; disjoint from evac cols)
            ones_ap = vb_sb[:].rearrange("p s (j w) -> p s j w", w=HD + 1)[
                :, :, :, HD : HD + 1
            ]
            nc.vector.memset(ones_ap, 1.0)

            def rope_proj(sc, which):
                # one 512-col chunk of rotated q^T or (split) k^T
                qs = bass.ts(sc, 512)
                wn, wpn, bn, bpn = (
                    ("wq", "wqp", "qb", "qbp")
                    if which == "q"
                    else ("wk", "wkp", "kb", "kbp")
                )
                ps1 = PPS.tile([128, 512], F32, tag="p1", name=f"p1_{which}{sc}")
                for c in range(4):
                    nc.tensor.matmul(
                        ps1,
                        w_sb[wn][:, c, :],
                        xt_sb[:, c, qs],
                        start=(c == 0),
                        stop=(c == 3),
                    )
                if qk_bias:
                    s1 = W.tile([128, 512], F32, tag="rope1")
                    nc.vector.tensor_scalar_add(s1, ps1, bias_sb[bn])
                else:
                    s1 = ps1
                t3 = W.tile([128, 512], F32, tag="rope3")
                nc.vector.tensor_tensor(out=t3, in0=s1, in1=cos_sb[:, qs], op=MUL)
                ps2 = PPS.tile([128, 512], F32, tag="p1", name=f"p2_{which}{sc}")
                for c in range(4):
                    nc.tensor.matmul(
                        ps2,
                        w_sb[wpn][:, c, :],
                        xt_sb[:, c, qs],
                        start=(c == 0),
                        stop=(c == 3),
                    )
                if qk_bias:
                    s2 = W.tile([128, 512], F32, tag="rope2")
                    nc.vector.tensor_scalar_add(s2, ps2, bias_sb[bpn])
                else:
                    s2 = ps2
                t4 = W.tile([128, 512], F32, tag="rope4")
                nc.vector.tensor_tensor(out=t4, in0=s2, in1=sin_sb[:, qs], op=MUL)
                if which == "q":
                    nc.vector.tensor_tensor(out=qr[:, qs], in0=t3, in1=t4, op=ADD)
                else:
                    nc.vector.tensor_tensor(
                        out=krA[0:HD, qs], in0=t3[0:HD, :], in1=t4[0:HD, :], op=ADD
                    )
                    nc.vector.tensor_tensor(
                        out=krB[HD:DPC, qs],
                        in0=t3[HD:DPC, :],
                        in1=t4[HD:DPC, :],
                        op=ADD,
                    )

            def v_proj(sc):
                for stl in range(4):
                    st = sc * 4 + stl
                    psv = PPS.tile([128, 128], F32, tag="p1", name=f"pv{st}")
                    for c in range(4):
                        nc.tensor.matmul(
                            psv,
                            xt_sb[:, c, bass.ds(sc * 512 + stl * 128, 128)],
                            w_sb["wv"][:, c, :],
                            start=(c == 0),
                            stop=(c == 3),
                        )
                    dsts = vb_sb[:, st, :].rearrange("p (j w) -> p j w", w=HD + 1)[
                        :, :, 0:HD
                    ]
                    if v_bias:
                        nc.vector.tensor_tensor(out=dsts, in0=psv, in1=vbias_bc, op=ADD)
                    else:
                        nc.vector.tensor_copy(out=dsts, in_=psv)

            zs = nc.dram_tensor("zscratch", [HPC, S], F32)

            # q chunk 0 first: scores need q before anything else.
            rope_proj(0, "q")

            for qt in range(NSC):
                qs = bass.ts(qt, 512)
                pso = [
                    POU.tile(
                        [HD + 1, 512], F32, tag="o", bufs=2, name=f"o{qt}_{h}"
                    )
                    for h in range(HPC)
                ]
                pt_hist = [[], []]
                for p in range(KC2):
                    if qt == 0 and p % 2 == 0:
                        # k/v projection chunk p//2 gates pairs 2*(p//2)..
                        sc = p // 2
                        rope_proj(sc, "k")
                        v_proj(sc)
                    if qt > 0 and p == 2 and qt + 1 < NSC:
                        # next q-chunk's projection as PE filler
                        rope_proj(qt + 1, "q")
                    if qt == 0 and p == 13:
                        rope_proj(1, "q")
                    for h in range(HPC):
                        krp = krA if h == 0 else krB
                        pss_t = PSS.tile(
                            [128, 1024], F32, tag="s", name=f"s{qt}_{p}_{h}"
                        )
                        # boost scores above all other PE work: they gate
                        # the exp stream (ScalarE is co-critical with PE)
                        with tc.high_priority(offset=20000):
                            for j in range(2):
                                kc = p * 2 + j
                                nc.tensor.matmul(
                                    pss_t[:, bass.ts(j, 512)],
                                    krp[:, bass.ts(kc, 128)],
                                    qr[:, qs],
                                    start=True,
                                    stop=True,
                                )
                        pt = W.tile(
                            [128, 1024],
                            BF16,
                            tag=f"pt{h}",
                            bufs=3,
                            name=f"pt{qt}_{p}_{h}",
                        )
                        nc.scalar.activation(out=pt, in_=pss_t, func=EXP, scale=0.125)
                        if p >= 2:
                            # PV deferred by TWO pairs: the PE queue never
                            # blocks on exp(p) or on the previous qt's pso
                            # being snapshotted out of PSUM
                            pd = p - 2
                            vcol = slice(h * (HD + 1), (h + 1) * (HD + 1))
                            for j in range(2):
                                kc = pd * 2 + j
                                nc.tensor.matmul(
                                    pso[h],
                                    vb_sb[:, kc, vcol],
                                    pt_hist[h][pd][:, bass.ts(j, 512)],
                                    start=(kc == 0),
                                    stop=False,
                                )
                        pt_hist[h].append(pt)
                for pd in (KC2 - 2, KC2 - 1):
                    for h in range(HPC):
                        vcol = slice(h * (HD + 1), (h + 1) * (HD + 1))
                        for j in range(2):
                            kc = pd * 2 + j
                            nc.tensor.matmul(
                                pso[h],
                                vb_sb[:, kc, vcol],
                                pt_hist[h][pd][:, bass.ts(j, 512)],
                                start=False,
                                stop=(kc == NKC - 1),
                            )
                # evacuate O' + Z, transpose Z via DRAM bounce, recip,
                # then per-head output projection scaled by 1/Z scalars
                stk = tc.high_priority(offset=-(10**6))
                stk.__enter__()
                for h in range(HPC):
                    hs = slice(h * HD, (h + 1) * HD)
                    otp = otA if h == 0 else otB
                    nc.vector.tensor_copy(out=otp[hs, qs], in_=pso[h][0:HD, :])
                    nc.vector.tensor_copy(
                        out=zrow[32 * h : 32 * h + 1, qs],
                        in_=pso[h][HD : HD + 1, :],
                    )
                for h in range(HPC):
                    nc.sync.dma_start(out=zs[h, qs], in_=zrow[32 * h : 32 * h + 1, qs])
                    nc.sync.dma_start(
                        out=zt[:, bass.ds(h * NUT + qt * 4, 4)],
                        in_=zs[h, qs].rearrange("(j p) -> p j", p=128),
                    )
                    nc.vector.reciprocal(
                        out=izt[:, bass.ds(h * NUT + qt * 4, 4)],
                        in_=zt[:, bass.ds(h * NUT + qt * 4, 4)],
                    )
                for utl in range(4):
                    ut = qt * 4 + utl
                    us = bass.ts(ut, 128)
                    if qt == NSC - 1:
                        # PSS banks are dead after the last exp: use them to
                        # unserialize the final output projections
                        wt = PSS.tile([128, 1024], F32, tag="s", name=f"uw{ut}")
                        psu = [wt[:, 0:DIM], wt[:, DIM : 2 * DIM]]
                    else:
                        psu = [
                            POU.tile([128, DIM], F32, tag="u", name=f"u{h}_{ut}")
                            for h in range(HPC)
                        ]
                    for h in range(HPC):
                        otp = otA if h == 0 else otB
                        nc.tensor.matmul(
                            psu[h], otp[:, us], wo_sb[:, :], start=True, stop=True
                        )
                    t_mid = W.tile([128, DIM], F32, tag="umid")
                    nc.vector.tensor_scalar_mul(t_mid, psu[0], izt[:, ut : ut + 1])
                    t_out = W.tile([128, DIM], F32, tag="uout")
                    nc.vector.scalar_tensor_tensor(
                        out=t_out,
                        in0=psu[1],
                        scalar=izt[:, NUT + ut : NUT + ut + 1],
                        in1=t_mid,
                        op0=MUL,
                        op1=ADD,
                    )
                    nc.sync.dma_start(out=out_e[us, :], in_=t_out)
                stk.__exit__(None, None, None)

    return nc


def _rope_tables():
    freqs = 10000.0 ** (-np.linspace(0.0, 1.0, HALF, endpoint=False))
    theta = np.arange(S, dtype=np.float64)[None, :] * freqs[:, None]  # [32, S]
    cos32 = np.cos(theta)
    sin32 = np.sin(theta)
    cosf = np.tile(np.concatenate([cos32, cos32], axis=0), (HPC, 1))
    sinf = np.tile(np.concatenate([-sin32, sin32], axis=0), (HPC, 1))
    return cosf, sinf


def kernel(x, wq_k, wq_b, wk_k, wk_b, wv_k, wv_b, wo_k, wo_b):
    from concourse.bass_utils import run_bass_kernel_spmd
    import ml_dtypes

    x = np.asarray(x, np.float32)
    wq_k = np.asarray(wq_k, np.float32)
    wq_b = np.asarray(wq_b, np.float32)
    wk_k = np.asarray(wk_k, np.float32)
    wk_b = np.asarray(wk_b, np.float32)
    wv_k = np.asarray(wv_k, np.float32)
    wv_b = np.asarray(wv_b, np.float32)
    wo_k = np.asarray(wo_k, np.float32)
    wo_b = np.asarray(wo_b, np.float32)

    qk_bias = bool(np.any(wq_b) or np.any(wk_b))
    v_bias = bool(np.any(wv_b))

    key = (qk_bias, v_bias)
    if key not in _CACHE:
        nc = _build(qk_bias, v_bias)
        _split_multiwait_drains(nc)
        _CACHE[key] = nc
    nc = _CACHE[key]

    mmdt = ml_dtypes.bfloat16

    cosf, sinf = _rope_tables()
    cosf = cosf.astype(np.float16)
    sinf = sinf.astype(np.float16)
    perm = np.r_[HALF:HD, 0:HALF]

    in_maps = []
    for c in range(NCORES):
        b = c // 4
        h0 = HPC * (c % 4)
        hsl = slice(h0, h0 + HPC)
        m = {
            "xt": np.ascontiguousarray(x[b].T).astype(mmdt),
            "wq": np.ascontiguousarray(wq_k[:, hsl, :].reshape(DIM, DPC)).astype(mmdt),
            "wqp": np.ascontiguousarray(wq_k[:, hsl, perm].reshape(DIM, DPC)).astype(mmdt),
            "wk": np.ascontiguousarray(wk_k[:, hsl, :].reshape(DIM, DPC)).astype(mmdt),
            "wkp": np.ascontiguousarray(wk_k[:, hsl, perm].reshape(DIM, DPC)).astype(mmdt),
            "wv": np.ascontiguousarray(wv_k[:, hsl, :].reshape(DIM, DPC)).astype(mmdt),
            "wo": np.ascontiguousarray(wo_k[hsl].reshape(DPC, DIM)).astype(mmdt),
            "cosf": cosf,
            "sinf": sinf,
            "zpage": np.zeros((1, 512), mmdt),
        }
        if qk_bias:
            m["qb"] = np.ascontiguousarray(wq_b[hsl].reshape(DPC, 1))
            m["qbp"] = np.ascontiguousarray(wq_b[hsl][:, perm].reshape(DPC, 1))
            m["kb"] = np.ascontiguousarray(wk_b[hsl].reshape(DPC, 1))
            m["kbp"] = np.ascontiguousarray(wk_b[hsl][:, perm].reshape(DPC, 1))
        if v_bias:
            m["vb"] = np.ascontiguousarray(wv_b[hsl].reshape(1, DPC))
        in_maps.append(m)

    res = run_bass_kernel_spmd(nc, in_maps, list(range(NCORES)))

    out = np.zeros((B, S, DIM), np.float32)
    for c in range(NCORES):
        out[c // 4] += res.results[c]["out"]
    out += wo_b[None, None, :]
    return out


# revision 10
# speedup vs baseline: 1.1588x; 1.1588x over previous
"""Trainium2 Bass kernel for nn_Attention_90074054132266.

Full multi-head attention (B=2, S=4096, D=512, H=8, HD=64) with RoPE on
q/k, sharded over 8 NeuronCores: batch x head-pair (data parallel over
batch, tensor parallel over heads; core c handles batch c//4, heads
2*(c%4), 2*(c%4)+1). Each core computes a partial output projection
(its 2 heads' contribution); the host sums the 4 per-batch partials
(the "all-reduce") and adds wo_b.

Per-core device algorithm (everything stored transposed, bf16 matmuls):
  - host passes x[b].T; projections q^T/k^T = wq^T-chunks @ x^T as
    N=512 bf16 matmuls. x^T is resident in SBUF (loaded once).
  - RoPE via duplicated projections with half-swapped weight columns
    (q2^T[d] = q^T[(d+32)%64 per head]) + sign-baked cos/sin tables
    (bf16): q_rot = q^T * cosf + q2^T * sinf  (3 VectorE ops).
  - scores computed transposed: S^T[k-chunk, q] = K_rot slice.T @ Q_rot
    (contraction over d=64, zero-padded per head to K=128 so the HAM
    clock gate keeps the PE at 2.4 GHz). exp(S/8) on ScalarE from PSUM
    with the 1/sqrt(hd) scale folded in (no max subtraction: scores
    ~ N(0,1), exp is safe in fp32).
  - P@V accumulates O'^T[65, q] over 32 k-chunks; V' has a ones column
    so row 64 = softmax denominator Z for free.
  - output projection U_h = O_h^T.T @ wo_h per head; normalization by
    1/Z_h[q] applied as a per-partition scalar on PSUM evacuation.

Pipeline structure (the point of this revision):
  - DMAs are emitted in consumption order (w, then per 512-chunk:
    xt, cos, sin) so the first projection starts ~3us in.
  - qt0's attention is interleaved with the k/v projection: pair p of
    k-chunks only needs k/v chunks from projection chunk p//2, so
    scores/exp/PV start after the first projection chunk instead of
    after all eight.
  - PV of pair p-1 is emitted AFTER scores of pair p: the PE queue is
    in-order, so this keeps the PE busy while ScalarE runs exp(p).
"""

import os
import sys

sys.path.insert(0, "/opt/trn_rl_repo")

import numpy as np

B, S, DIM, HEADS, HD = 2, 4096, 512, 8, 64
HALF = HD // 2
NCORES = 8
HPC = 2  # heads per core
DPC = HPC * HD  # 128 projection columns per core
NSC = S // 512  # 8 column chunks of 512
NKC = S // 128  # 32 k-chunks of 128
NUT = S // 128  # 32 q-row tiles of 128
KC2 = NKC // 2  # 16 pairs of k-chunks (exp batches of [128, 1024])
VW = 2 * (HD + 1)  # 130: per-k V' row for both heads (64+1 each)

_CACHE = {}


def _split_multiwait_drains(nc):
    """The walrus build in this container rejects any instruction with
    more than one sync-wait ("Too many sync wait commands"). Hoist the
    extra waits onto preceding same-engine NoOps, leaving one wait on
    the original instruction."""
    import bass_rust
    import concourse.mybir as mybir

    for fn in nc.m.functions:
        for bb in fn.blocks:
            new_insts = []
            changed = False
            for inst in bb.instructions:
                si = getattr(inst, "sync_info", None)
                if si is not None and len(si.on_wait) > 1:
                    waits = list(si.on_wait)
                    for k, w in enumerate(waits[:-1]):
                        d = mybir.InstNoOp(name=f"{inst.name}w{k}", ins=[], outs=[])
                        d.engine = inst.engine
                        d.sync_info = bass_rust.SyncInfo(on_wait=[w], on_update=[])
                        new_insts.append(d)
                    inst.sync_info = bass_rust.SyncInfo(
                        on_wait=[waits[-1]], on_update=list(si.on_update)
                    )
                    changed = True
                new_insts.append(inst)
            if changed:
                bb.instructions = new_insts


def _build(qk_bias, v_bias):
    import concourse.bass as bass
    import concourse.tile as tile
    from concourse import mybir

    F32 = mybir.dt.float32
    BF16 = mybir.dt.bfloat16
    F16 = mybir.dt.float16
    EXP = mybir.ActivationFunctionType.Exp
    MUL = mybir.AluOpType.mult
    ADD = mybir.AluOpType.add

    nc = bass.Bass("TRN2")

    xt_e = nc.declare_dram_parameter("xt", [DIM, S], BF16, isOutput=False)
    w_e = {}
    for name in ("wq", "wqp", "wk", "wkp", "wv"):
        w_e[name] = nc.declare_dram_parameter(name, [DIM, DPC], BF16, isOutput=False)
    wo_e = nc.declare_dram_parameter("wo", [DPC, DIM], BF16, isOutput=False)
    cos_e = nc.declare_dram_parameter("cosf", [DPC, S], F16, isOutput=False)
    sin_e = nc.declare_dram_parameter("sinf", [DPC, S], F16, isOutput=False)
    b_e = {}
    if qk_bias:
        for name in ("qb", "qbp", "kb", "kbp"):
            b_e[name] = nc.declare_dram_parameter(name, [DPC, 1], F32, isOutput=False)
    if v_bias:
        b_e["vb"] = nc.declare_dram_parameter("vb", [1, DPC], F32, isOutput=False)
    z_e = nc.declare_dram_parameter("zpage", [1, 512], BF16, isOutput=False)
    out_e = nc.declare_dram_parameter("out", [S, DIM], F32, isOutput=True)

    with tile.TileContext(nc) as tc:
        with (
            tc.tile_pool(name="persist", bufs=1) as P,
            tc.tile_pool(name="work", bufs=2) as W,
            tc.tile_pool(name="pps", bufs=1, space="PSUM") as PPS,
            tc.tile_pool(name="pss", bufs=2, space="PSUM") as PSS,
            tc.tile_pool(name="pou", bufs=1, space="PSUM") as POU,
        ):
            # ---- persistent SBUF tensors ----
            qr = P.tile([DPC, S], BF16, tag="qr")  # rotated q^T
            # rotated k^T, zero-padded per head to full K=128 contraction
            # (row-masked K=64 matmuls don't count as PE-busy for the HAM
            # clock gate; mixing them with PV pins the PE at 1.2 GHz)
            krA = P.tile([DPC, S], BF16, tag="krA")
            krB = P.tile([DPC, S], BF16, tag="krB")

            def zfill(dst):
                srcap = bass.AP(
                    tensor=z_e[:].tensor,
                    offset=z_e[:].offset,
                    ap=[[0, HD], [0, NSC], [1, 512]],
                )
                nc.sync.dma_start(out=dst, in_=srcap)
            # V' rows: per k-chunk st, V[k, :] for head A cols 0:64 + ones
            # col 64, head B cols 65:129 + ones col 129.
            vb_sb = P.tile([128, NKC, VW], BF16, tag="vboth")
            # unnormalized O^T, zero-padded per head (HAM: keep K=128)
            otA = P.tile([DPC, S], BF16, tag="otA")
            otB = P.tile([DPC, S], BF16, tag="otB")
            wo_sb = P.tile([DPC, DIM], BF16, tag="wo")
            zrow = P.tile([33, S], F32, tag="zrow")
            zt = P.tile([128, 2 * NUT], F32, tag="zt")
            izt = P.tile([128, 2 * NUT], F32, tag="izt")
            # resident x^T: [p, dim-chunk, col]
            xt_sb = P.tile([128, 4, S], BF16, tag="xt")
            cos_sb = P.tile([DPC, S], F16, tag="cos")
            sin_sb = P.tile([DPC, S], F16, tag="sin")
            w_sb = {}
            for name in ("wq", "wqp", "wk", "wkp", "wv"):
                t = P.tile([128, 4, DPC], BF16, tag=name, name=f"w_{name}")
                w_sb[name] = t

            # ---- DMA emission in strict first-need order (all on the
            # SP engine: only its queues are HW-DGE; triggers serialize
            # per transfer, so order = data arrival order) ----
            def wload(name):
                nc.sync.dma_start(
                    out=w_sb[name],
                    in_=w_e[name][:].rearrange("(c p) m -> p c m", p=128),
                )

            xt_r = xt_e[:].rearrange("(c p) s -> c p s", p=128)

            def xtload(sc, w=512):
                qs_ = bass.ds(sc * 512, w)
                for c in range(4):
                    nc.sync.dma_start(out=xt_sb[:, c, qs_], in_=xt_r[c, :, qs_])

            def csload(sc, w=512):
                qs_ = bass.ds(sc * 512, w)
                nc.sync.dma_start(out=cos_sb[:, qs_], in_=cos_e[:, qs_])
                nc.sync.dma_start(out=sin_sb[:, qs_], in_=sin_e[:, qs_])

            wload("wq")
            xtload(0)
            wload("wqp")
            csload(0)
            bias_sb = {}
            if qk_bias:
                for name in ("qb", "qbp", "kb", "kbp"):
                    t = P.tile([DPC, 1], F32, tag=name, name=f"b_{name}")
                    nc.sync.dma_start(out=t, in_=b_e[name][:])
                    bias_sb[name] = t
            if v_bias:
                vbias_bc = P.tile([128, DPC], F32, tag="vbias")
                src = bass.AP(
                    tensor=b_e["vb"].tensor,
                    offset=b_e["vb"].offset,
                    ap=[[0, 128], [1, DPC]],
                )
                nc.sync.dma_start(out=vbias_bc, in_=src)

            wload("wk")
            wload("wkp")
            zfill(krA[HD:DPC, :])
            zfill(krB[0:HD, :])
            wload("wv")
            zfill(otA[HD:DPC, :])
            zfill(otB[0:HD, :])
            xtload(1)
            csload(1)
            nc.sync.dma_start(out=wo_sb, in_=wo_e[:])
            for sc in range(2, NSC):
                xtload(sc)
                csload(sc)

            # ones columns of V' (written once; disjoint from evac cols)
            ones_ap = vb_sb[:].rearrange("p s (j w) -> p s j w", w=HD + 1)[
                :, :, :, HD : HD + 1
            ]
            nc.vector.memset(ones_ap, 1.0)

            def rope_proj(sc, which):
                # one 512-col chunk of rotated q^T or (split) k^T
                qs = bass.ts(sc, 512)
                wn, wpn, bn, bpn = (
                    ("wq", "wqp", "qb", "qbp")
                    if which == "q"
                    else ("wk", "wkp", "kb", "kbp")
                )
                ps1 = PPS.tile([128, 512], F32, tag="p1", name=f"p1_{which}{sc}")
                for c in range(4):
                    nc.tensor.matmul(
                        ps1,
                        w_sb[wn][:, c, :],
                        xt_sb[:, c, qs],
                        start=(c == 0),
                        stop=(c == 3),
                    )
                if qk_bias:
                    s1 = W.tile([128, 512], F32, tag="rope1")
                    nc.vector.tensor_scalar_add(s1, ps1, bias_sb[bn])
                else:
                    s1 = ps1
                t3 = W.tile([128, 512], F32, tag="rope3")
                nc.vector.tensor_tensor(out=t3, in0=s1, in1=cos_sb[:, qs], op=MUL)
                ps2 = PPS.tile([128, 512], F32, tag="p1", name=f"p2_{which}{sc}")
                for c in range(4):
                    nc.tensor.matmul(
                        ps2,
                        w_sb[wpn][:, c, :],
                        xt_sb[:, c, qs],
                        start=(c == 0),
                        stop=(c == 3),
                    )
                if qk_bias:
                    s2 = W.tile([128, 512], F32, tag="rope2")
                    nc.vector.tensor_scalar_add(s2, ps2, bias_sb[bpn])
                else:
                    s2 = ps2
                t4 = W.tile([128, 512], F32, tag="rope4")
                nc.vector.tensor_tensor(out=t4, in0=s2, in1=sin_sb[:, qs], op=MUL)
                if which == "q":
                    nc.vector.tensor_tensor(out=qr[:, qs], in0=t3, in1=t4, op=ADD)
                else:
                    nc.vector.tensor_tensor(
                        out=krA[0:HD, qs], in0=t3[0:HD, :], in1=t4[0:HD, :], op=ADD
                    )
                    nc.vector.tensor_tensor(
                        out=krB[HD:DPC, qs],
                        in0=t3[HD:DPC, :],
                        in1=t4[HD:DPC, :],
                        op=ADD,
                    )

            def v_proj(sc):
                for stl in range(4):
                    st = sc * 4 + stl
                    psv = PPS.tile([128, 128], F32, tag="p1", name=f"pv{st}")
                    for c in range(4):
                        nc.tensor.matmul(
                            psv,
                            xt_sb[:, c, bass.ds(sc * 512 + stl * 128, 128)],
                            w_sb["wv"][:, c, :],
                            start=(c == 0),
                            stop=(c == 3),
                        )
                    dsts = vb_sb[:, st, :].rearrange("p (j w) -> p j w", w=HD + 1)[
                        :, :, 0:HD
                    ]
                    if v_bias:
                        nc.vector.tensor_tensor(out=dsts, in0=psv, in1=vbias_bc, op=ADD)
                    else:
                        nc.vector.tensor_copy(out=dsts, in_=psv)

            zs = nc.dram_tensor("zscratch", [HPC, S], F32)

            # q chunk 0 first: scores need q before anything else.
            rope_proj(0, "q")

            for qt in range(NSC):
                qs = bass.ts(qt, 512)
                pso = [
                    POU.tile(
                        [HD + 1, 512], F32, tag="o", bufs=2, name=f"o{qt}_{h}"
                    )
                    for h in range(HPC)
                ]
                pt_hist = [[], []]
                for p in range(KC2):
                    if qt == 0 and p % 2 == 0:
                        # k/v projection chunk p//2 gates pairs 2*(p//2)..
                        sc = p // 2
                        rope_proj(sc, "k")
                        v_proj(sc)
                    if qt > 0 and p == 2 and qt + 1 < NSC:
                        # next q-chunk's projection as PE filler
                        rope_proj(qt + 1, "q")
                    if qt == 0 and p == 13:
                        rope_proj(1, "q")
                    for h in range(HPC):
                        krp = krA if h == 0 else krB
                        pss_t = PSS.tile(
                            [128, 1024], F32, tag="s", name=f"s{qt}_{p}_{h}"
                        )
                        # boost scores above all other PE work: they gate
                        # the exp stream (ScalarE is co-critical with PE)
                        with tc.high_priority(offset=20000):
                            for j in range(2):
                                kc = p * 2 + j
                                nc.tensor.matmul(
                                    pss_t[:, bass.ts(j, 512)],
                                    krp[:, bass.ts(kc, 128)],
                                    qr[:, qs],
                                    start=True,
                                    stop=True,
                                )
                        pt = W.tile(
                            [128, 1024],
                            BF16,
                            tag=f"pt{h}",
                            bufs=3,
                            name=f"pt{qt}_{p}_{h}",
                        )
                        nc.scalar.activation(out=pt, in_=pss_t, func=EXP, scale=0.125)
                        if p >= 2:
                            # PV deferred by TWO pairs: the PE queue never
                            # blocks on exp(p) or on the previous qt's pso
                            # being snapshotted out of PSUM
                            pd = p - 2
                            vcol = slice(h * (HD + 1), (h + 1) * (HD + 1))
                            for j in range(2):
                                kc = pd * 2 + j
                                nc.tensor.matmul(
                                    pso[h],
                                    vb_sb[:, kc, vcol],
                                    pt_hist[h][pd][:, bass.ts(j, 512)],
                                    start=(kc == 0),
                                    stop=False,
                                )
                        pt_hist[h].append(pt)
                for pd in (KC2 - 2, KC2 - 1):
                    for h in range(HPC):
                        vcol = slice(h * (HD + 1), (h + 1) * (HD + 1))
                        for j in range(2):
                            kc = pd * 2 + j
                            nc.tensor.matmul(
                                pso[h],
                                vb_sb[:, kc, vcol],
                                pt_hist[h][pd][:, bass.ts(j, 512)],
                                start=False,
                                stop=(kc == NKC - 1),
                            )
                # evacuate O' + Z, transpose Z via DRAM bounce, recip,
                # then per-head output projection scaled by 1/Z scalars
                stk = tc.high_priority(offset=-(10**6))
                stk.__enter__()
                for h in range(HPC):
                    hs = slice(h * HD, (h + 1) * HD)
                    otp = otA if h == 0 else otB
                    nc.vector.tensor_copy(out=otp[hs, qs], in_=pso[h][0:HD, :])
                    nc.vector.tensor_copy(
                        out=zrow[32 * h : 32 * h + 1, qs],
                        in_=pso[h][HD : HD + 1, :],
                    )
                for h in range(HPC):
                    nc.sync.dma_start(out=zs[h, qs], in_=zrow[32 * h : 32 * h + 1, qs])
                    nc.sync.dma_start(
                        out=zt[:, bass.ds(h * NUT + qt * 4, 4)],
                        in_=zs[h, qs].rearrange("(j p) -> p j", p=128),
                    )
                    nc.vector.reciprocal(
                        out=izt[:, bass.ds(h * NUT + qt * 4, 4)],
                        in_=zt[:, bass.ds(h * NUT + qt * 4, 4)],
                    )
                for utl in range(4):
                    ut = qt * 4 + utl
                    us = bass.ts(ut, 128)
                    if qt == NSC - 1:
                        # PSS banks are dead after the last exp: use them to
                        # unserialize the final output projections
                        wt = PSS.tile([128, 1024], F32, tag="s", name=f"uw{ut}")
                        psu = [wt[:, 0:DIM], wt[:, DIM : 2 * DIM]]
                    else:
                        psu = [
                            POU.tile([128, DIM], F32, tag="u", name=f"u{h}_{ut}")
                            for h in range(HPC)
                        ]
                    for h in range(HPC):
                        otp = otA if h == 0 else otB
                        nc.tensor.matmul(
                            psu[h], otp[:, us], wo_sb[:, :], start=True, stop=True
                        )
                    t_mid = W.tile([128, DIM], F32, tag="umid")
                    nc.vector.tensor_scalar_mul(t_mid, psu[0], izt[:, ut : ut + 1])
                    t_out = W.tile([128, DIM], F32, tag="uout")
                    nc.vector.scalar_tensor_tensor(
                        out=t_out,
                        in0=psu[1],
                        scalar=izt[:, NUT + ut : NUT + ut + 1],
                        in1=t_mid,
                        op0=MUL,
                        op1=ADD,
                    )
                    nc.sync.dma_start(out=out_e[us, :], in_=t_out)
                stk.__exit__(None, None, None)

    return nc


def _rope_tables():
    freqs = 10000.0 ** (-np.linspace(0.0, 1.0, HALF, endpoint=False))
    theta = np.arange(S, dtype=np.float64)[None, :] * freqs[:, None]  # [32, S]
    cos32 = np.cos(theta)
    sin32 = np.sin(theta)
    cosf = np.tile(np.concatenate([cos32, cos32], axis=0), (HPC, 1))
    sinf = np.tile(np.concatenate([-sin32, sin32], axis=0), (HPC, 1))
    return cosf, sinf


def kernel(x, wq_k, wq_b, wk_k, wk_b, wv_k, wv_b, wo_k, wo_b):
    from concourse.bass_utils import run_bass_kernel_spmd
    import ml_dtypes

    x = np.asarray(x, np.float32)
    wq_k = np.asarray(wq_k, np.float32)
    wq_b = np.asarray(wq_b, np.float32)
    wk_k = np.asarray(wk_k, np.float32)
    wk_b = np.asarray(wk_b, np.float32)
    wv_k = np.asarray(wv_k, np.float32)
    wv_b = np.asarray(wv_b, np.float32)
    wo_k = np.asarray(wo_k, np.float32)
    wo_b = np.asarray(wo_b, np.float32)

    qk_bias = bool(np.any(wq_b) or np.any(wk_b))
    v_bias = bool(np.any(wv_b))

    key = (qk_bias, v_bias)
    if key not in _CACHE:
        nc = _build(qk_bias, v_bias)
        _split_multiwait_drains(nc)
        _CACHE[key] = nc
    nc = _CACHE[key]

    mmdt = ml_dtypes.bfloat16

    cosf, sinf = _rope_tables()
    cosf = cosf.astype(np.float16)
    sinf = sinf.astype(np.float16)
    perm = np.r_[HALF:HD, 0:HALF]

    in_maps = []
    for c in range(NCORES):
        b = c // 4
        h0 = HPC * (c % 4)
        hsl = slice(h0, h0 + HPC)
        m = {
            "xt": np.ascontiguousarray(x[b].T).astype(mmdt),
            "wq": np.ascontiguousarray(wq_k[:, hsl, :].reshape(DIM, DPC)).astype(mmdt),
            "wqp": np.ascontiguousarray(wq_k[:, hsl, perm].reshape(DIM, DPC)).astype(mmdt),
            "wk": np.ascontiguousarray(wk_k[:, hsl, :].reshape(DIM, DPC)).astype(mmdt),
            "wkp": np.ascontiguousarray(wk_k[:, hsl, perm].reshape(DIM, DPC)).astype(mmdt),
            "wv": np.ascontiguousarray(wv_k[:, hsl, :].reshape(DIM, DPC)).astype(mmdt),
            "wo": np.ascontiguousarray(wo_k[hsl].reshape(DPC, DIM)).astype(mmdt),
            "cosf": cosf,
            "sinf": sinf,
            "zpage": np.zeros((1, 512), mmdt),
        }
        if qk_bias:
            m["qb"] = np.ascontiguousarray(wq_b[hsl].reshape(DPC, 1))
            m["qbp"] = np.ascontiguousarray(wq_b[hsl][:, perm].reshape(DPC, 1))
            m["kb"] = np.ascontiguousarray(wk_b[hsl].reshape(DPC, 1))
            m["kbp"] = np.ascontiguousarray(wk_b[hsl][:, perm].reshape(DPC, 1))
        if v_bias:
            m["vb"] = np.ascontiguousarray(wv_b[hsl].reshape(1, DPC))
        in_maps.append(m)

    res = run_bass_kernel_spmd(nc, in_maps, list(range(NCORES)))

    out = np.zeros((B, S, DIM), np.float32)
    for c in range(NCORES):
        out[c // 4] += res.results[c]["out"]
    out += wo_b[None, None, :]
    return out
